# revision 20
# baseline (speedup 1.0000x reference)
"""TRN2 Bass kernel for nn_MetaBaseline (DN4-style local-descriptor kNN).

Reference computation (per batch b):
  q = normalize(input1[b].reshape(75*100, 640), axis=-1)       # query patches
  s = normalize(input2[b].reshape(2500, 640), axis=-1)         # support descs
  scores = q @ s.T                                             # [7500, 2500]
  per way group w (columns [500w, 500w+500)): top-k per row, mean over k,
  then sum over the 100 patches of each query -> out [75, 5].

Sharding: data-parallel over (b, query-quarter): 8 cores, each handles one
batch's quarter of queries (19 queries padded) with that batch's full
support replicated (per the sharding hint).

V4 architecture (fp8 PE stream + ACT-assisted DVE top-k):
- Host prep (part of the shard/replicate step, as in V3): support features
  L2-normalized, scaled x16, cast to fp8 e4m3; queries cast to fp8 raw (a
  positive per-row scale can't change that row's top-k; 1/(k*16*|q_p|) is
  folded into the host-built indicator matrix). NEW in V4: DRAM layouts are
  way-major (sT) / m-tile-major (qT) so every DMA descriptor is a large
  contiguous per-partition block (640B-3200B instead of 320-640B), and the
  critical first-unit bytes are fetched in parallel on 3 queues with plain
  FIFO ordering (no dummy-op delay choreography needed).
- Scores: per (way, m-tile): 2 DoubleRow fp8 matmuls + 1 plain fp8 matmul
  accumulate [128, 500] into one PSUM bank. Measured warm cadence is 211ns
  per FD=500 matmul (DR == plain) -> 633ns/unit, 75 units ~ 47.5us. This is
  the fp8 hardware floor (no fp8 DoublePixel/uint8 path exists in bass).
- Top-k: DVE max8. NEW: for 4 of 5 ways the Scalar engine (ACT, otherwise
  idle) first evicts the PSUM scores to SBUF bf16 (~710ns, fully parallel);
  max8 from SBUF bf16 measures ~583ns vs ~687-720ns from PSUM fp32 (the
  init/access-latency difference - HW-measured), so the DVE chain drops to
  ~2.93us per m-tile < the PE's 3.17us and the kernel becomes PE-paced.
- Finale: NEW: no per-m-tile DVE fold at all. The per-m-tile matmul uses
  the raw top-8 tile mxs[m] [128, 5*8] bf16 as the moving operand with the
  indicator slice [128, 19] stationary, accumulating out_ps [19, 40] across
  m-tiles (delayed one m-tile so the PE never waits on a fresh max8; FD=40
  matmuls issue in ~25-50ns). One windowed DVE tensor_reduce at the very
  end folds [19, 5, :k] -> [19, 5], which also discards the unused top-k+1..8
  slots. Epilogue: DMA [19, 5] out; host gather needs no transpose.
- PE warmup (HAM clock ramp) unchanged: ~34 dependence-free FD=128 matmuls
  cover the ~3.4us cold window while the critical DMA lands.
"""
import os
from contextlib import ExitStack

import numpy as np
import ml_dtypes

import concourse.bass as bass  # noqa: F401
import concourse.mybir as mybir
import concourse.tile as tile
from concourse import bacc
from concourse.bass_utils import run_bass_kernel_spmd

# Problem geometry (hardcoded per contest rules)
B, Q, WAY, SHOT, H, W, C = 2, 75, 5, 5, 10, 10, 640
HW = H * W               # 100 patches per query / support image
NQ = 19                  # queries per core (4 cores x 19 = 76 >= 75)
MT = 15                  # patch M-tiles of 128 -> 1920 rows (1900 real)
PAD_P = MT * 128
NS = WAY * SHOT * HW     # 2500 support descriptors per batch
KC = 5                   # C chunks of 128 (640 = 5*128)
P = 128
NW = SHOT * HW           # 500 support descriptors per way group
N_CORES = 8
N_WARM = int(os.environ.get("N_WARM", "40"))
ACT_WAYS = (0, 1, 2, 3)  # ways whose scores ACT evicts to bf16 for max8

FP8 = ml_dtypes.float8_e4m3

_prog_cache: dict[int, object] = {}


def _build(k: int):
    """Build + compile the per-core SPMD program for neighbor_k == k."""
    assert 1 <= k <= 8, f"neighbor_k={k} not supported (need 1..8)"
    nc = bacc.Bacc("TRN2", target_bir_lowering=False, debug=False)
    f32 = mybir.dt.float32
    bf16 = mybir.dt.bfloat16
    fp8 = mybir.dt.float8e4
    DR = mybir.MatmulPerfMode.DoubleRow

    # m-tile-major query bands: [P, m, c, i] (contiguous 640B/partition per
    # m-tile), way-major support bands: [P, w, c, j] (2500B/partition per way)
    qT_d = nc.dram_tensor("qT", [P, MT * KC * P], fp8, kind="ExternalInput").ap()
    sT_d = nc.dram_tensor("sT", [P, WAY * KC * NW], fp8,
                          kind="ExternalInput").ap()
    ind_d = nc.dram_tensor("ind", [P, MT * NQ], bf16, kind="ExternalInput").ap()
    out_d = nc.dram_tensor("out", [NQ, WAY], f32, kind="ExternalOutput").ap()

    with tile.TileContext(nc) as tc:
        with ExitStack() as ctx:
            const = ctx.enter_context(tc.tile_pool(name="const", bufs=1))
            big = ctx.enter_context(tc.tile_pool(name="big", bufs=1))
            stg = ctx.enter_context(tc.tile_pool(name="stg", bufs=4))
            mxp = ctx.enter_context(tc.tile_pool(name="mxp", bufs=MT))
            outp = ctx.enter_context(
                tc.tile_pool(name="outp", bufs=1, space="PSUM")
            )
            spp = ctx.enter_context(
                tc.tile_pool(name="spp", bufs=7, space="PSUM")
            )

            qT = big.tile([P, MT * KC * P], fp8, name="qT")
            sT = big.tile([P, WAY * KC * NW], fp8, name="sT")
            ind_sb = const.tile([P, MT * NQ], bf16, name="ind_sb")
            qT4 = qT.rearrange("p (m c i) -> p m c i", m=MT, c=KC)
            sT4 = sT.rearrange("p (w c j) -> p w c j", w=WAY, c=KC)

            out_ps = outp.tile([NQ, WAY * 8], f32)

            # ---- PE warmup (HAM clock ramp; no DMA deps) ----
            wtile = const.tile([P, P], fp8, name="wtile")
            nc.vector.memset(wtile, 1.0)
            for i in range(N_WARM):
                wps = spp.tile([P, NW], f32, tag="psc", name=f"w{i}")
                nc.tensor.matmul(wps[:, 0:P], wtile, wtile,
                                 start=True, stop=True)

            # ---- DMAs ----
            # The 16 DMA engines round-robin across ALL queues' rings, so
            # per-queue FIFO alone does not protect the critical set: a
            # queue that finishes early starts its next (non-critical) DMA
            # and steals bandwidth chip-wide (HW-observed: critical-ready
            # jitter 10.5-15.7us). Fix: only the critical set (sT w0 on
            # sync, qT m0 then m1-4 on scalar) is dep-free; every later DMA
            # is gated behind an ACT probe that reads the w0 band (RAW) and
            # pre-writes one byte into that DMA's destination (WAW), so the
            # Tile deps serialize it after the critical transfers complete.
            # Transfer rate is descriptor-size-bound (~60KB/us at 640B/part,
            # ~200-290KB/us at 2500-3200B/part) -> multi-m-tile blocks.
            # sync:   sT w0 | qT m5-9 | sT w1 | sT w3
            # scalar: qT m0 | qT m1-4 | (probes) | qT m10-14 | sT w2 | sT w4
            WB = KC * NW  # way band size (2500)
            MB = KC * P   # m-tile band size (640)
            nc.sync.dma_start(out=sT[:, 0:WB], in_=sT_d[:, 0:WB])
            nc.scalar.dma_start(out=qT[:, 0:MB], in_=qT_d[:, 0:MB])
            nc.scalar.dma_start(out=qT[:, MB:5 * MB], in_=qT_d[:, MB:5 * MB])
            nc.gpsimd.dma_start(out=ind_sb, in_=ind_d)
            # probes: gate each non-critical DMA region behind qT m0 (done
            # ~9.6us; gated DMAs then pay ~2us DGE restart latency, landing
            # from ~12us, still well before their deadlines)
            for off, tens in ((5 * MB, qT), (7 * MB, qT), (10 * MB, qT),
                              (WB, sT), (2 * WB, sT), (3 * WB, sT),
                              (4 * WB, sT)):
                nc.scalar.copy(tens[:, off:off + 1], qT[:, 0:1])
            nc.sync.dma_start(out=qT[:, 5 * MB:7 * MB],
                              in_=qT_d[:, 5 * MB:7 * MB])
            nc.sync.dma_start(out=qT[:, 7 * MB:10 * MB],
                              in_=qT_d[:, 7 * MB:10 * MB])
            nc.scalar.dma_start(out=qT[:, 10 * MB:MT * MB],
                                in_=qT_d[:, 10 * MB:MT * MB])
            nc.sync.dma_start(out=sT[:, WB:2 * WB], in_=sT_d[:, WB:2 * WB])
            nc.scalar.dma_start(out=sT[:, 2 * WB:3 * WB],
                                in_=sT_d[:, 2 * WB:3 * WB])
            nc.sync.dma_start(out=sT[:, 3 * WB:4 * WB],
                              in_=sT_d[:, 3 * WB:4 * WB])
            nc.scalar.dma_start(out=sT[:, 4 * WB:5 * WB],
                                in_=sT_d[:, 4 * WB:5 * WB])

            # ---- main loop: way-outer, m-tile inner ----
            mxs = [None] * MT
            for w in range(WAY):
                for m in range(MT):
                    if w == 0:
                        mxs[m] = mxp.tile([P, WAY * 8], bf16, tag="mx",
                                          name=f"mx{m}")
                    psc = spp.tile([P, NW + 1], f32, tag="psc",
                                   name=f"psc{m}_{w}")
                    for i in range(2):
                        nc.tensor.matmul(
                            psc[:, 0:NW],
                            qT4[:, m, 2 * i:2 * i + 2, :],
                            sT4[:, w, 2 * i:2 * i + 2, :],
                            start=(i == 0),
                            stop=False,
                            perf_mode=DR,
                        )
                    nc.tensor.matmul(
                        psc[:, 0:NW],
                        qT4[:, m, 4, :],
                        sT4[:, w, 4, :],
                        start=False,
                        stop=True,
                    )
                    if w == 0 and m < 5:
                        # pipeline fill: the first two units' max8 read PSUM
                        # directly (saves the ~1.3us ACT-copy fill latency;
                        # hoist-safe: nothing precedes them on the DVE queue)
                        nc.vector.max(mxs[m][:, w * 8:(w + 1) * 8],
                                      psc[:, 0:NW])
                    elif w in ACT_WAYS:
                        sg = stg.tile([P, NW], bf16, tag="stg",
                                      name=f"sg{m}_{w}")
                        nc.scalar.copy(sg, psc[:, 0:NW])
                        nc.vector.max(mxs[m][:, w * 8:(w + 1) * 8], sg)
                    else:
                        # direct-PSUM max8, but gated through a tiny ACT op
                        # (writes a -1e30 sentinel into the spare column) so
                        # every max8 waits on the monotone Activation sem --
                        # without this the Tile scheduler hoists the lone
                        # PE-dependent max8 ahead of ACT-path ones, causing
                        # a multi-us DVE head-of-line stall (HW-observed).
                        nc.scalar.activation(
                            psc[:, NW:NW + 1], psc[:, 0:1],
                            mybir.ActivationFunctionType.Copy,
                            scale=0.0, bias=-1e30)
                        nc.vector.max(mxs[m][:, w * 8:(w + 1) * 8], psc)
            # finale: indicator-stationary matmuls with the raw top-8 tiles
            # as the (cheap, FD=40) moving operand, batched AFTER the unit
            # stream: inside the way-4 pass each of these costs ~260ns of
            # PE pipeline (stationary switch + un-hidable small MM,
            # HW-measured 910ns/unit cadence); back-to-back at the end they
            # run at ~150ns each and overlap the trailing max8s.
            for m in range(MT):
                nc.tensor.matmul(
                    out_ps, ind_sb[:, m * NQ:(m + 1) * NQ], mxs[m],
                    start=(m == 0), stop=(m == MT - 1))

            # ---- epilogue: fold top-k columns, DMA [19, 5] out ----
            out_k = const.tile([NQ, WAY], f32, name="out_k")
            nc.vector.tensor_reduce(
                out_k,
                out_ps.rearrange("q (w j) -> q w j", w=WAY)[:, :, 0:k],
                axis=mybir.AxisListType.X,
                op=mybir.AluOpType.add,
            )
            nc.sync.dma_start(out=out_d, in_=out_k)

    nc.compile()
    return nc


def get_program(k: int):
    if k not in _prog_cache:
        _prog_cache[k] = _build(k)
    return _prog_cache[k]


def make_in_maps(input1: np.ndarray, input2: np.ndarray, k: int):
    """Shard full inputs into per-core input maps.

    Host side of the shard/replicate step: fp8 cast + m-tile-major band
    transpose of q; L2-normalize+scale+cast+way-major band transpose of the
    replicated support features; indicator matrix with the per-patch-row
    1/(k*16*|q_p|) factor folded in.
    """
    input1 = np.asarray(input1, dtype=np.float32)
    input2 = np.asarray(input2, dtype=np.float32)
    in_maps = []
    for core in range(N_CORES):
        b = core // 4
        qs = (core % 4) * NQ
        qe = min(Q, qs + NQ)
        nq = qe - qs
        qdat = input1[b].reshape(Q, HW, C)[qs:qe].reshape(-1, C)
        qfull = np.ones((PAD_P, C), np.float32)
        qfull[: nq * HW] = qdat
        # qT m-tile-major bands: [P, m, c, i]; value = qfull[m*128+i, c*128+p]
        q8 = qfull.astype(FP8)  # [1920, 640]
        qT = np.ascontiguousarray(
            q8.reshape(MT, P, KC, P).transpose(3, 0, 2, 1).reshape(
                P, MT * KC * P))
        # support: normalize, scale x16 into fp8 range, way-major bands:
        # [P, w, c, j]; value = s_n[w*500+j, c*128+p]
        sdat = input2[b].reshape(NS, C)
        s_n = (16.0 * sdat / np.linalg.norm(sdat, axis=1, keepdims=True)
               ).astype(FP8)
        sT = np.ascontiguousarray(
            s_n.reshape(WAY, NW, KC, P).transpose(3, 0, 2, 1).reshape(
                P, WAY * KC * NW))
        # indicator: patch row p of M-tile t belongs to query (t*128+p)//HW,
        # pre-scaled by 1/(k * 16 * |q_row|) (fp8-consistent norms)
        qn = np.linalg.norm(q8.astype(np.float32), axis=1)
        ind = np.zeros((P, MT * NQ), np.float32)
        g = np.arange(MT * P)
        j = g // HW
        valid = j < nq
        ind[g[valid] % P, (g[valid] // P) * NQ + j[valid]] = (
            1.0 / (k * 16.0 * qn[g[valid]]))
        in_maps.append({
            "qT": qT, "sT": sT,
            "ind": ind.astype(ml_dtypes.bfloat16),
        })
    return in_maps


def gather_out(results) -> np.ndarray:
    out = np.zeros((B, Q, WAY), np.float32)
    for core in range(N_CORES):
        b = core // 4
        qs = (core % 4) * NQ
        n = min(Q, qs + NQ) - qs
        out[b, qs:qs + n] = results[core]["out"][:n]
    return out


def kernel(input1, input2, neighbor_k):
    k = int(np.asarray(neighbor_k))
    nc = get_program(k)
    in_maps = make_in_maps(input1, input2, k)
    # the axon-tunneled device occasionally reports a transient
    # "unrecoverable" state right after a previous process's teardown;
    # it recovers within seconds, so retry a couple of times
    import time
    last = None
    for attempt in range(3):
        try:
            res = run_bass_kernel_spmd(
                nc, in_maps, core_ids=list(range(N_CORES)))
            return gather_out(res.results)
        except Exception as e:  # noqa: BLE001
            last = e
            if attempt < 2:
                time.sleep(20.0 * (attempt + 1))
    raise last
